# revision 14
# baseline (speedup 1.0000x reference)
"""Trainium2 Bass kernel for MultiHeadAttention with relative position bias.

Reference computation (B=2, S=2048, D=1024, H=16, Dk=64, MAX_REL=128):
    Q,K,V = x@W{q,k,v}.T + b      (per-head reshape)
    scores = QK^T/sqrt(Dk) + rel_bias_matrix
    out = softmax(scores) @ V, heads merged, @ Wo.T + bo

Sharding (8 cores): core c handles batch b=c//4 and 4 heads hg=4*(c%4)..+4
(data + head parallel). Q/K/V projections column-split per head group,
Wo row-split; the partial outputs are summed on the host (the "all-reduce").

Per-core device algorithm (all matmul operands bf16; f32 PSUM accumulate):
  xT (1024,2048) -> Q^T,K^T (c_local=256, S) on PE; V as (S, 256).
  Per head pair (row-tiled 64x128 PE, two heads concurrent):
    S^T[k,q] = K^T.T @ Q^T, then P^T = exp(S^T/8) via one ACT pass; the
    "future" region (q-k <= -128) is fixed with a constant multiply and the
    384-wide Toeplitz band with a host-precomputed exp(bias - c_past) tile
    (both DVE, bf16).  P^T stored bf16.
  PV: lhsT is the M=128 augmented [V_h0 | ones] (hh=0) / [ones | V_h1]
  (hh=1), so acc rows carry the head output AND the softmax denominator
  replicated across 64 partitions (PV matmuls are N-bound, so this is free).
  Normalize (per 512-q strip): two partition-shifting PSUM->SBUF copies
  collect both heads' denominators, reciprocal_approx_fast (DVE), then two
  fused evict-normalize multiplies into bf16 ct.
  Wo partial = ct.T @ (Wo^T rows), per-q normalization already applied.

The hard wall is 4 heads x S^2 = 16.8M exps/core on the scalar engine
(~110us at 1 elem/cycle/lane), so the issue order keeps ACT saturated:
K flight, Q(t0), then attention with the V projection interleaved into the
first block's QK stream (PV deferred 8 chunks), Q(t1) between blocks (next
block's QK proceeds on the other PSUM pool), Wo first half interleaved into
the third block (overlapping its output DMA with attention), remainder as
tail.  Startup input DMAs and tail output DMAs alternate between the two
hardware DGE queues (SP + Activation).
"""

import math
import os
import sys

for _p in ("/opt/trn_rl_repo", "/root/.axon_site", "/root/.axon_site/_ro/trn_rl_repo",
           "/root/.axon_site/_ro/pypackages"):
    if os.path.isdir(_p) and _p not in sys.path:
        sys.path.append(_p)

import numpy as np
import ml_dtypes

import concourse.bass as bass
import concourse.mybir as mybir
import concourse.tile as tile
from concourse import bacc
from contextlib import ExitStack

# Problem constants (hardcoded per the contract).
B, S, D = 2, 2048, 1024
H, DK = 16, 64
MAX_REL = 128
N_CORES = 8
CORES_PER_BATCH = 4
HEADS_PER_CORE = H // CORES_PER_BATCH  # 4
CL = HEADS_PER_CORE * DK               # 256 local channels
N_PAIRS = HEADS_PER_CORE // 2          # 2 head pairs
QH = 1024                              # q processed in halves
N_QH = S // QH                         # 2
N_KC = S // 128                        # 16 k chunks
BAND = 3 * 128                         # band width in q for one k chunk
NDC = D // 128                         # 8 contraction chunks

F32 = mybir.dt.float32
BF16 = mybir.dt.bfloat16

SCALE = 1.0 / math.sqrt(DK)

EXP = mybir.ActivationFunctionType.Exp


def build_program(reps=1):
    nc = bacc.Bacc("TRN2", target_bir_lowering=False, debug=False)

    xt_d = nc.declare_dram_parameter("xt", [D, S], BF16, isOutput=False)
    wqt_d = nc.declare_dram_parameter("wqt", [D, CL], BF16, isOutput=False)
    wkt_d = nc.declare_dram_parameter("wkt", [D, CL], BF16, isOutput=False)
    wvt_d = nc.declare_dram_parameter("wvt", [D, CL], BF16, isOutput=False)
    wot_d = nc.declare_dram_parameter("wot", [CL, D], BF16, isOutput=False)
    bqk_d = nc.declare_dram_parameter("bqk", [128, 4], F32, isOutput=False)
    band_d = nc.declare_dram_parameter("band", [128, HEADS_PER_CORE, BAND], BF16,
                                       isOutput=False)
    # per-head activation constants, replicated over partitions:
    # [:, 2h] = exp(c_fut - c_past) multiplier
    abias_d = nc.declare_dram_parameter("abias", [128, 2 * HEADS_PER_CORE], F32,
                                        isOutput=False)
    out_d = nc.declare_dram_parameter("out_p", [S, D], F32, isOutput=True)

    with tile.TileContext(nc) as tc, ExitStack() as ctx:
        # ---------- long-lived SBUF ----------
        persist = ctx.enter_context(tc.tile_pool(name="persist", bufs=1))
        q_sb = persist.tile([128, 2, S], BF16, tag="q_sb")
        k_sb = persist.tile([128, 2, S], BF16, tag="k_sb")
        # per (kc, pair): [V_h0(64) | ones(128) | V_h1(64)]
        v_sb = persist.tile([128, N_KC, N_PAIRS, 256], BF16, tag="v_sb")
        ct_sb = persist.tile([128, 2, S], BF16, tag="ct_sb")
        wo_sb = persist.tile([128, 2, D], BF16, tag="wo_sb")
        band_sb = persist.tile([128, HEADS_PER_CORE, BAND], BF16, tag="band_sb")
        bqk_sb = persist.tile([128, 4], F32, tag="bqk_sb")
        abias_sb = persist.tile([128, 2 * HEADS_PER_CORE], F32, tag="abias_sb")

        xw = ctx.enter_context(tc.tile_pool(name="xw", bufs=1))
        xt_sb = xw.tile([128, NDC, S], BF16, tag="xt_sb")
        wq_sb = xw.tile([128, NDC, CL], BF16, tag="wq_sb")
        wk_sb = xw.tile([128, NDC, CL], BF16, tag="wk_sb")
        wv_sb = xw.tile([128, NDC, CL], BF16, tag="wv_sb")

        # ---------- PSUM pools ----------
        stp = ctx.enter_context(tc.tile_pool(name="stp", bufs=2, space="PSUM"))
        accp = ctx.enter_context(tc.tile_pool(name="accp", bufs=2, space="PSUM"))

        # ---------- small pools ----------
        outp = ctx.enter_context(tc.tile_pool(name="outp", bufs=4))
        nrm = ctx.enter_context(tc.tile_pool(name="nrm", bufs=4))
        ptp = ctx.enter_context(tc.tile_pool(name="ptp", bufs=10))

        sb = dict(q=q_sb, k=k_sb, v=v_sb, ct=ct_sb, wo=wo_sb, band=band_sb,
                  bqk=bqk_sb, abias=abias_sb, xt=xt_sb, wq=wq_sb, wk=wk_sb,
                  wv=wv_sb)
        dram = dict(xt=xt_d, wqt=wqt_d, wkt=wkt_d, wvt=wvt_d, wot=wot_d,
                    bqk=bqk_d, band=band_d, abias=abias_d, out=out_d)
        pools = dict(stp=stp, accp=accp, outp=outp, nrm=nrm, ptp=ptp)

        for rep in range(reps):
            _phases(nc, tc, sb, dram, pools, rep)

    nc.compile()
    return nc


def _phases(nc, tc, sb, dram, pools, rep):
    q_sb, k_sb, v_sb, ct_sb, wo_sb = sb["q"], sb["k"], sb["v"], sb["ct"], sb["wo"]
    band_sb, bqk_sb, abias_sb = sb["band"], sb["bqk"], sb["abias"]
    xt_sb, wq_sb, wk_sb, wv_sb = sb["xt"], sb["wq"], sb["wk"], sb["wv"]
    stp, accp, outp, nrm, ptp = (pools[n] for n in
                                 ("stp", "accp", "outp", "nrm", "ptp"))

    GROUPS = (range(0, NDC // 2), range(NDC // 2, NDC))
    xt_v = dram["xt"].ap().rearrange("(c p) s -> p c s", p=128)

    # ---------- input DMAs: two HWDGE queues, K-flight needs first ----------
    q2 = (nc.sync, nc.scalar)
    nc.sync.dma_start(out=wk_sb, in_=dram["wkt"].ap().rearrange("(c p) m -> p c m", p=128))
    nc.scalar.dma_start(out=bqk_sb, in_=dram["bqk"].ap())
    for dc in range(NDC):
        q2[dc % 2].dma_start(out=xt_sb[:, dc, :], in_=xt_v[:, dc, :])
    nc.scalar.dma_start(out=wq_sb, in_=dram["wqt"].ap().rearrange("(c p) m -> p c m", p=128))
    nc.sync.dma_start(out=wv_sb, in_=dram["wvt"].ap().rearrange("(c p) m -> p c m", p=128))
    nc.scalar.dma_start(out=wo_sb, in_=dram["wot"].ap().rearrange("(c p) m -> p c m", p=128))
    nc.sync.dma_start(out=abias_sb, in_=dram["abias"].ap())
    nc.sync.dma_start(out=band_sb, in_=dram["band"].ap())
    # ones blocks of the augmented V (middle 128 columns of each pair block)
    nc.vector.memset(v_sb[:, :, :, 64:192], 1.0)

    # ---------- self-contained PE "pieces" (slot + matmuls + evict) --------
    # Each piece is <= ~1.7us of PE work and releases its PSUM slot at the
    # end, so pieces can be interleaved one-per-QK-round inside attention
    # without stalling the in-order PE queue or deadlocking the tile rings.
    def piece_qk(w_sb, o_sb, bcol, j, t, half):
        """One [128,512] slice of a Q/K projection: 8 matmuls + DVE evict."""
        slot = stp.tile([128, 1024], F32, tag="st", name="pq_slot")
        c0 = t * 1024 + half * 512
        for dc in range(NDC):
            nc.tensor.matmul(
                slot[:, 0:512],
                lhsT=w_sb[:, dc, j * 128:(j + 1) * 128],
                rhs=xt_sb[:, dc, c0:c0 + 512],
                start=(dc == 0), stop=(dc == NDC - 1),
            )
        nc.vector.tensor_scalar_add(
            out=o_sb[:, j, c0:c0 + 512],
            in0=slot[:, 0:512],
            scalar1=bqk_sb[:, bcol + j:bcol + j + 1],
        )

    def piece_v(sc):
        """V projection for one s-chunk: 8 matmuls + 2 DVE evicts."""
        slot = stp.tile([128, 1024], F32, tag="st", name="pv_slot")
        for dc in range(NDC):
            nc.tensor.matmul(
                slot[:, 0:CL],
                lhsT=xt_sb[:, dc, sc * 128:(sc + 1) * 128],
                rhs=wv_sb[:, dc, :],
                start=(dc == 0), stop=(dc == NDC - 1),
            )
        src = slot[:, 0:CL].rearrange("p (hp dd) -> p hp dd", hp=2)
        # even heads -> cols 0:64, odd heads -> cols 192:256 of pair block
        nc.vector.tensor_copy(out=v_sb[:, sc, :, 0:64], in_=src[:, :, 0:64])
        nc.vector.tensor_copy(out=v_sb[:, sc, :, 192:256], in_=src[:, :, 64:128])

    # ---------- attention ----------
    def qk_exp_fix(pair, hh, kc, w0, st, pt_dst):
        """QK matmuls + exp + band/future fixups for one (head, chunk)."""
        k0 = kc * 128
        h = 2 * pair + hh
        p0 = hh * 64
        for half in range(QH // 512):
            nc.tensor.matmul(
                st[:, half * 512:(half + 1) * 512],
                lhsT=k_sb[p0:p0 + 64, pair, k0:k0 + 128],
                rhs=q_sb[p0:p0 + 64, pair,
                         w0 + half * 512:w0 + (half + 1) * 512],
                start=True, stop=True,
                tile_position=(p0, 0),
            )
        nc.scalar.activation(out=pt_dst, in_=st, func=EXP, scale=SCALE)
        # future region (q <= k0-129): multiply by exp(c_fut - c_past)
        fut_end = min(max(k0 - 128, w0), w0 + QH)
        n_fut = fut_end - w0
        if n_fut > 0:
            nc.vector.tensor_scalar_mul(
                out=pt_dst[:, 0:n_fut], in0=pt_dst[:, 0:n_fut],
                scalar1=abias_sb[:, 2 * h:2 * h + 1],
            )
        # band: q in [k0-128, k0+256) -> multiply exp(bias - c_past)
        b_lo = max(k0 - 128, w0)
        b_hi = min(k0 + 2 * 128, w0 + QH)
        if b_hi > b_lo:
            m0 = b_lo - (k0 - 128)
            nc.vector.tensor_mul(
                out=pt_dst[:, b_lo - w0:b_hi - w0],
                in0=pt_dst[:, b_lo - w0:b_hi - w0],
                in1=band_sb[:, h, m0:m0 + (b_hi - b_lo)],
            )

    def qk_round(pair, kc, w0):
        pt = ptp.tile([128, 2, QH], BF16, tag="pt", name="pt")
        for hh in range(2):
            st = stp.tile([128, QH], F32, tag="st", name="st")
            qk_exp_fix(pair, hh, kc, w0, st, pt[:, hh, :])
        return pt

    def pv_round(pair, kc, pt, accs):
        for hh in range(2):
            for sub in range(QH // 512):
                nc.tensor.matmul(
                    accs[hh][:, sub * 512:(sub + 1) * 512],
                    lhsT=v_sb[:, kc, pair, hh * 128:(hh + 1) * 128],
                    rhs=pt[:, hh, sub * 512:(sub + 1) * 512],
                    start=(kc == 0), stop=(kc == N_KC - 1),
                )

    def normalize(pair, w0, acc_a, acc_b):
        """Per 512-q strip: shift-copy denominators, approx-recip, fused
        evict-normalize multiplies (acc rows: see attn_block docstring)."""
        for sp in range(2):
            c0 = sp * 512
            den = nrm.tile([128, 512], F32, tag="den", name="den")
            rden = nrm.tile([128, 512], F32, tag="rden", name="rden")
            nc.vector.tensor_copy(out=den[0:64, :], in_=acc_a[64:128, c0:c0 + 512])
            nc.vector.tensor_copy(out=den[64:128, :], in_=acc_b[0:64, c0:c0 + 512])
            nc.vector.reciprocal_approx_fast(out=rden, in_=den)
            nc.vector.tensor_mul(
                out=ct_sb[0:64, pair, w0 + c0:w0 + c0 + 512],
                in0=acc_a[0:64, c0:c0 + 512], in1=rden[0:64, :],
            )
            nc.vector.tensor_mul(
                out=ct_sb[64:128, pair, w0 + c0:w0 + c0 + 512],
                in0=acc_b[64:128, c0:c0 + 512], in1=rden[64:128, :],
            )

    def piece_wo(st_i, dma_eng):
        ps = stp.tile([128, 1024], F32, tag="st", name="wo_ps")
        o_sb = outp.tile([128, D], F32, tag="o_sb", name="o_sb")
        for j in range(2):
            for mt in range(2):
                nc.tensor.matmul(
                    ps[:, mt * 512:(mt + 1) * 512],
                    lhsT=ct_sb[:, j, st_i * 128:(st_i + 1) * 128],
                    rhs=wo_sb[:, j, mt * 512:(mt + 1) * 512],
                    start=(j == 0), stop=(j == 1),
                )
        nc.vector.tensor_copy(out=o_sb, in_=ps)
        dma_eng.dma_start(out=dram["out"].ap()[st_i * 128:(st_i + 1) * 128, :],
                          in_=o_sb)

    def attn_block(pair, qh, fillers=(), defer=2):
        """One (head-pair, q-half): QK+exp+fix rounds with one filler piece
        interleaved per round, PV deferred `defer` rounds, then normalize.

        acc_a (hh=0) rows: 0:64 = ct_h0 unnormalized, 64:128 = den_h0 x64.
        acc_b (hh=1) rows: 0:64 = den_h1 x64, 64:128 = ct_h1 unnormalized.
        """
        w0 = qh * QH
        acc_a = accp.tile([128, QH], F32, tag="acc", name="acc_a")
        acc_b = accp.tile([128, QH], F32, tag="acc", name="acc_b")
        accs = [acc_a, acc_b]
        pending = []
        for kc in range(N_KC):
            pt = qk_round(pair, kc, w0)
            if kc < len(fillers):
                for f in fillers[kc]:
                    f()
            pending.append((kc, pt))
            if len(pending) > defer:
                k2, pt2 = pending.pop(0)
                pv_round(pair, k2, pt2, accs)
        for k2, pt2 in pending:
            pv_round(pair, k2, pt2, accs)
        normalize(pair, w0, acc_a, acc_b)

    # ---------- schedule ----------
    # Pre-attention: only the j=0 (pair 0) halves of K/Q at t=0 -- the
    # minimum for block (0,0)'s exp stream to start.  Everything else rides
    # inside attention as fillers, budgeted ~<=1.7us of PE work per round.
    K = lambda j, t, h: (lambda: piece_qk(wk_sb, k_sb, 2, j, t, h))
    Q = lambda j, t, h: (lambda: piece_qk(wq_sb, q_sb, 0, j, t, h))
    V = lambda sc: (lambda: piece_v(sc))
    W = lambda i: (lambda: piece_wo(i, nc.sync))

    for h in range(2):
        piece_qk(wk_sb, k_sb, 2, 0, 0, h)
    for h in range(2):
        piece_qk(wq_sb, q_sb, 0, 0, 0, h)

    # blk00: all 16 V chunks (PV(kc) needs V(kc), defer=2 covers it), K t1
    # for pair 0 (needed from kc8), then pair 1's K/Q t0 (needed by blk10).
    fill00 = [[V(0)], [V(1), K(0, 1, 0)], [V(2)], [V(3), K(0, 1, 1)],
              [V(4)], [V(5)], [V(6)], [V(7)],
              [V(8)], [V(9), K(1, 0, 0)], [V(10)], [V(11), K(1, 0, 1)],
              [V(12)], [V(13), Q(1, 0, 0)], [V(14)], [V(15), Q(1, 0, 1)]]
    attn_block(0, 0, fill00)
    # blk10: K t1 for pair 1 (needed from its kc8), Q t1 for both pairs
    # (needed by blk01/blk11).
    fill10 = [[K(1, 1, 0)], [], [K(1, 1, 1)], [],
              [Q(0, 1, 0)], [], [Q(0, 1, 1)], [],
              [Q(1, 1, 0)], [], [Q(1, 1, 1)]]
    attn_block(1, 0, fill10)
    # blk01: Wo chunks for q 0:1024 (ct ready since blk10's normalize);
    # their output DMAs overlap the remaining attention.
    fill01 = [[W(0)], [W(1)], [W(2)], [W(3)], [W(4)], [W(5)], [W(6)], [W(7)]]
    attn_block(0, 1, fill01)
    attn_block(1, 1)
    # Wo tail: q rows 1024:2048, output DMAs alternate between both queues
    for st_i in range(8, 16):
        piece_wo(st_i, (nc.sync, nc.scalar)[st_i % 2])


def make_core_inputs(x, Wq, bq, Wk, bk, Wv, bv, Wo, bo, rel_bias):
    """Host-side shard prep. Returns list of 8 in_maps."""
    bf16 = ml_dtypes.bfloat16
    x = np.asarray(x, np.float32)
    in_maps = []
    WqT = np.ascontiguousarray(np.asarray(Wq, np.float32).T)
    WkT = np.ascontiguousarray(np.asarray(Wk, np.float32).T)
    WvT = np.ascontiguousarray(np.asarray(Wv, np.float32).T)
    WoT = np.ascontiguousarray(np.asarray(Wo, np.float32).T)
    rel = np.asarray(rel_bias, np.float32)
    xt = [np.ascontiguousarray(x[b].T).astype(bf16) for b in range(B)]

    # band multiplier: [p, h_local, m] = exp(bias(q,k) - c_past), q-k = m-128-p
    p_i = np.arange(128)[:, None]
    m_i = np.arange(BAND)[None, :]
    delta = np.clip(m_i - 128 - p_i, -MAX_REL, MAX_REL) + MAX_REL  # [128, 384]

    for c in range(N_CORES):
        b = c // CORES_PER_BATCH
        g = c % CORES_PER_BATCH
        c0 = g * CL
        heads = np.arange(g * HEADS_PER_CORE, (g + 1) * HEADS_PER_CORE)

        bqk = np.empty((128, 4), np.float32)
        bqk[:, 0] = np.asarray(bq, np.float32)[c0:c0 + 128]
        bqk[:, 1] = np.asarray(bq, np.float32)[c0 + 128:c0 + 256]
        bqk[:, 2] = np.asarray(bk, np.float32)[c0:c0 + 128]
        bqk[:, 3] = np.asarray(bk, np.float32)[c0 + 128:c0 + 256]

        band = np.empty((128, HEADS_PER_CORE, BAND), np.float32)
        abias = np.empty((128, 2 * HEADS_PER_CORE), np.float32)
        for i, hg in enumerate(heads):
            c_past = rel[hg, 2 * MAX_REL]
            band[:, i, :] = np.exp(rel[hg][delta] - c_past)
            abias[:, 2 * i] = np.exp(rel[hg, 0] - c_past)  # future multiplier
            abias[:, 2 * i + 1] = c_past
        in_maps.append({
            "xt": xt[b],
            "wqt": np.ascontiguousarray(WqT[:, c0:c0 + CL]).astype(bf16),
            "wkt": np.ascontiguousarray(WkT[:, c0:c0 + CL]).astype(bf16),
            "wvt": np.ascontiguousarray(WvT[:, c0:c0 + CL]).astype(bf16),
            "wot": np.ascontiguousarray(WoT[c0:c0 + CL, :]).astype(bf16),
            "bqk": bqk,
            "band": band.astype(bf16),
            "abias": abias,
        })
    return in_maps


_NC_CACHE = {}


def get_program(**kw):
    key = tuple(sorted(kw.items()))
    if key not in _NC_CACHE:
        _NC_CACHE[key] = build_program(**kw)
    return _NC_CACHE[key]


def kernel(x, Wq, bq, Wk, bk, Wv, bv, Wo, bo, rel_bias):
    from concourse.bass_utils import run_bass_kernel_spmd

    nc = get_program()
    in_maps = make_core_inputs(x, Wq, bq, Wk, bk, Wv, bv, Wo, bo, rel_bias)
    res = run_bass_kernel_spmd(nc, in_maps, core_ids=list(range(N_CORES)))
    results = res.results

    Wo_np = np.asarray(Wo, np.float32)
    const = np.asarray(bv, np.float32) @ Wo_np.T + np.asarray(bo, np.float32)
    out = np.zeros((B, S, D), np.float32)
    for c in range(N_CORES):
        out[c // CORES_PER_BATCH] += results[c]["out_p"]
    out += const[None, None, :]
    return out
